# revision 77
# baseline (speedup 1.0000x reference)
"""BuddyPool kernel for Trainium2 (Bass/Tile), 8-core data-parallel.

Problem: cue (64,5,1024), patches (64,32,32,1024) ->
  sim = einsum('bkd,bhwd->bkhw'); idx = argmax(sim over hw);
  roi = mean of boundary-clamped 3x3 patch window around idx  -> (64,5,1024)

Sharding: batch across 8 cores, 8 samples/core. Inside one core:
  - stream patches[s] per d-chunk ([128 hw-part, 8 c, 128 d] slices, 512B
    descriptors) so PE transposes + sim matmuls pipeline inside the load
  - PE-transpose 128x128 tiles -> patchesT [d, hw]; sim matmul accumulates
    cueT.T @ patchesT over the 8 d-chunks
  - argmax via DVE max/max_index (first-max tie rule == jnp.argmax)
  - 3x3 window realized ON-CHIP: row/col membership masks rm/cm [K,32]
    (normalized by clamped span), wgt = rm (x) cm outer product [K, HW],
    PE-transposed to wgtT [HW, K]
  - roi = wgtT.T @ patches_nat, accumulated over the 8 hw-chunks, using the
    natural-layout patches still in SBUF -> zero DRAM gather traffic
  - per-sample chain (argmax -> masks -> wgt -> roi) for sample s-1 is
    interleaved at fixed issue points inside sample s's front stream
"""

import sys

if "/opt/trn_rl_repo" not in sys.path:
    sys.path.insert(0, "/opt/trn_rl_repo")

import numpy as np

import concourse.bass as bass
import concourse.tile as tile
from concourse import mybir
from concourse.masks import make_identity

P = 128
B = 64          # full batch
NCORES = 8
NS = B // NCORES  # samples per core
K = 5
D = 1024
H = W = 32
HW = H * W      # 1024
NDC = D // P    # 8 d-chunks
NHWC = HW // P  # 8 hw-chunks
F32 = mybir.dt.float32
F32R = mybir.dt.float32r
U32 = mybir.dt.uint32
BF16 = mybir.dt.bfloat16

TRANS_DT = F32R   # nat/pt tiles + transpose PSUM dtype
IDENT_DT = F32R   # walrus rejects bf16 ident x f32r data (32/non-32 mix)
LAG_D = 2         # sim matmuls lag transposes by this many d-chunks
NAT_BUFS = 4
PT_BUFS = 4
MAX_WAITS = 1
PRIO_BUMP = 0     # extra scheduler priority (lateness) for sim matmuls
WAIT_DCS = 3      # build-time scheduling: delay sim matmuls by N chunk-times
WAIT_PER_S = 12500  # build-time scheduling: per-sample slope for the delay
N_WARM = 30       # PE keep-warm dummy transposes in the epilogue
WAIT_T0 = None    # optional per-sample front-start times for the hints
DVE_COPY_DCS = 0  # how many trailing dcs route their h1 pt copy to DVE
COPY_PIN = 0      # ns offset for pinning pt copies to the ideal timeline
WARM_OFF = 9      # warmers' schedule-hint offset in chunk-times
FIRST_ON_ACT = False  # ACT preamble is longer + SEQ head-of-line: keep loads on SP

ALU = mybir.AluOpType


def split_multiwait_ctrl(nc, max_waits=1):
    """Walrus (neuronxcc CoreV3) rejects instructions carrying more than
    one sync wait. Hoist excess waits onto same-engine NOPs emitted just
    before the instruction — program order on the engine's sequencer makes
    this semantically identical (waits are a conjunction)."""
    n_split = 0
    for fn in nc.m.functions:
        for bb in fn.blocks:
            new_list = []
            for inst in bb.instructions:
                si = inst.sync_info
                lim = 1 if isinstance(inst, mybir.InstMatmult) else max_waits
                if si is not None and si.on_wait and len(si.on_wait) > lim:
                    waits = list(si.on_wait)
                    extra, keep = waits[:-lim], waits[-lim:]
                    for i, w in enumerate(extra):
                        d = mybir.InstNoOp(
                            name=f"{inst.name}-ws{i}",
                            engine=inst.engine,
                            ins=[],
                            outs=[],
                            sync_info=mybir.SyncInfo(on_wait=[w], on_update=[]),
                        )
                        nc.register_instruction(d)
                        new_list.append(d)
                    si.on_wait = keep
                    n_split += 1
                new_list.append(inst)
            bb.instructions[:] = new_list
    return n_split


def build_bass():
    nc = bass.Bass(
        trn_type="TRN2",
        target_bir_lowering=False,
        debug=False,
        enable_asserts=False,
    )

    cue_d = nc.dram_tensor("cue", [NS * K, D], F32, kind="ExternalInput").ap()
    pat_d = nc.dram_tensor("patches", [NS * HW, D], F32, kind="ExternalInput").ap()
    out_d = nc.dram_tensor("out", [NS * K, D], F32, kind="ExternalOutput").ap()

    with tile.TileContext(nc) as tc:
        build_kernel(tc, out_d, cue_d, pat_d)
    split_multiwait_ctrl(nc, max_waits=MAX_WAITS)
    return nc


def build_kernel(tc, out_d, cue_d, pat_d):
    nc = tc.nc
    from contextlib import ExitStack

    ctx = ExitStack()
    const = ctx.enter_context(tc.tile_pool(name="const", bufs=1))
    natp = ctx.enter_context(tc.tile_pool(name="nat", bufs=NAT_BUFS))
    ptp = ctx.enter_context(tc.tile_pool(name="pt", bufs=PT_BUFS))
    smallp = ctx.enter_context(tc.tile_pool(name="small", bufs=2))
    outp = ctx.enter_context(tc.tile_pool(name="outp", bufs=2))
    pst = ctx.enter_context(tc.tile_pool(name="ps_t", bufs=3, space="PSUM"))
    pss = ctx.enter_context(tc.tile_pool(name="ps_s", bufs=2, space="PSUM"))
    psrT = ctx.enter_context(tc.tile_pool(name="ps_rt", bufs=1, space="PSUM"))

    # ---- sample 0 patch loads ahead of everything (SP queue = loads only) --
    nat_tiles = {}

    def issue_loads(s):
        nat = natp.tile([P, NHWC, D], TRANS_DT, tag="nat")
        nat_tiles[s] = nat
        src = pat_d[s * HW : (s + 1) * HW, :].rearrange(
            "(c p) d -> p c d", p=P
        ).bitcast(TRANS_DT)
        for dc in range(NDC):
            # very first chunk goes out on the ACT queue: its sequencer
            # clears the framework preamble earlier than SP's, so the
            # whole (gapless) load stream starts sooner
            eng = nc.scalar if (s == 0 and dc == 0 and FIRST_ON_ACT) else nc.sync
            eng.dma_start(
                out=nat[:, :, dc * P : (dc + 1) * P],
                in_=src[:, :, dc * P : (dc + 1) * P],
            )
        return nat

    issue_loads(0)

    # ---- constants ----
    ident_f = const.tile([P, P], F32)
    make_identity(nc, ident_f[:])
    ident = const.tile([P, P], IDENT_DT)
    nc.vector.tensor_copy(out=ident[:], in_=ident_f[:])

    # iota pair [K, 2, 32]: row 0 = 0..31 (h grid), row 1 = 0..31 (w grid)
    io_u = const.tile([K, 2, 32], U32)
    nc.gpsimd.iota(io_u[:], pattern=[[0, 2], [1, 32]], channel_multiplier=0)
    io2 = const.tile([K, 2, 32], F32)
    nc.vector.tensor_copy(out=io2[:], in_=io_u[:])

    # ---- cue -> cueT ----
    cue_sb = const.tile([NS * K, D], F32)
    nc.scalar.dma_start(out=cue_sb[:], in_=cue_d[:])
    ident_cue = const.tile([NS * K, NS * K], F32)
    make_identity(nc, ident_cue[:])
    cueT = const.tile([P, NDC, NS * K], TRANS_DT)
    for dc in range(NDC):
        ps = pst.tile([P, 512], F32, tag="pst")
        nc.tensor.transpose(
            out=ps[:, : NS * K],
            in_=cue_sb[:, dc * P : (dc + 1) * P],
            identity=ident_cue[:],
        )
        nc.vector.tensor_copy(out=cueT[:, dc, :], in_=ps[:, : NS * K])

    # ------------------------------------------------------------------
    # Per-sample stages. chain/roi of sample s-1 are interleaved into
    # sample s's front stream at fixed points (after dc 1,2,3,4) so no
    # engine queue head-of-line blocks on the serial argmax chain.
    # ------------------------------------------------------------------
    state = {}  # s -> dict with sim_ps etc.

    def stage_chain_a(s):
        """argmax + index math + masks + wgt outer product (all DVE);
        max/max_index read the sim PSUM directly (validated on HW)."""
        st = state[s]
        sim_ps = st["sim_ps"]
        mx8 = smallp.tile([K, 8], F32, tag="mx8")
        idx8 = smallp.tile([K, 8], U32, tag="idx8")
        nc.vector.max(out=mx8[:], in_=sim_ps[:])
        nc.vector.max_index(out=idx8[:], in_max=mx8[:], in_values=sim_ps[:])

        # h = idx >> 5, w = idx & 31 (exact in u32), then -> f32
        # (walrus rejects u32-in/f32-out bit ops, so the convert is separate)
        sc = smallp.tile([K, 8], F32, tag="sc")
        hw_u = smallp.tile([K, 2], U32, tag="hwu")
        nc.vector.tensor_scalar(
            out=hw_u[:, 0:1], in0=idx8[:, 0:1], scalar1=5, scalar2=None,
            op0=ALU.logical_shift_right,
        )
        nc.vector.tensor_scalar(
            out=hw_u[:, 1:2], in0=idx8[:, 0:1], scalar1=31, scalar2=None,
            op0=ALU.bitwise_and,
        )
        nc.vector.tensor_copy(out=sc[:, 0:2], in_=hw_u[:])

        # membership masks for rows+cols in one shot:
        #   m2[:, 0, :] = (|iota - h| <= 1), m2[:, 1, :] = (|iota - w| <= 1)
        t64 = smallp.tile([K, 2, 32], F32, tag="t64")
        nc.vector.tensor_tensor(
            out=t64[:],
            in0=io2[:],
            in1=sc[:, 0:2].unsqueeze(2).broadcast_to((K, 2, 32)),
            op=ALU.subtract,
        )
        # membership = (clamp(d, -1, 1) == d)  (abs_max/is_le is not a
        # valid DVE op pair on HW)
        m2 = smallp.tile([K, 2, 32], F32, tag="m2")
        nc.vector.tensor_scalar(
            out=m2[:], in0=t64[:], scalar1=-1.0, scalar2=1.0,
            op0=ALU.max, op1=ALU.min,
        )
        nc.vector.tensor_tensor(
            out=m2[:], in0=m2[:], in1=t64[:], op=ALU.is_equal
        )
        if s == NS - 1:
            # Drain-only short form (the steady-state stream must not
            # change: the scheduler equilibrium is sensitive to it):
            # 1/span = 1/3 + (1/6) * (x==0 | x==31)  — exact for H=W=32
            nc.vector.tensor_scalar(
                out=sc[:, 2:4], in0=sc[:, 0:2], scalar1=0.0, scalar2=None,
                op0=ALU.is_equal,
            )
            nc.vector.tensor_scalar(
                out=sc[:, 4:6], in0=sc[:, 0:2], scalar1=float(H - 1),
                scalar2=None, op0=ALU.is_equal,
            )
            nc.vector.tensor_tensor(
                out=sc[:, 2:4], in0=sc[:, 2:4], in1=sc[:, 4:6], op=ALU.add
            )
            nc.vector.tensor_scalar(
                out=sc[:, 6:8], in0=sc[:, 2:4], scalar1=1.0 / 6.0,
                scalar2=1.0 / 3.0, op0=ALU.mult, op1=ALU.add,
            )
        else:
            # clamped spans: span = min(x+1,31) - max(x-1,0) + 1; recip
            nc.vector.tensor_scalar(
                out=sc[:, 2:4], in0=sc[:, 0:2], scalar1=1.0,
                scalar2=float(H - 1), op0=ALU.add, op1=ALU.min,
            )
            nc.vector.tensor_scalar(
                out=sc[:, 4:6], in0=sc[:, 0:2], scalar1=-1.0, scalar2=0.0,
                op0=ALU.add, op1=ALU.max,
            )
            nc.vector.tensor_tensor(
                out=sc[:, 2:4], in0=sc[:, 2:4], in1=sc[:, 4:6],
                op=ALU.subtract
            )
            nc.vector.tensor_scalar(
                out=sc[:, 2:4], in0=sc[:, 2:4], scalar1=1.0, scalar2=None,
                op0=ALU.add,
            )
            nc.vector.reciprocal(out=sc[:, 6:8], in_=sc[:, 2:4])
        # normalized masks, then wgt = rm (x) cm -> [K, H, W]
        nc.vector.tensor_tensor(
            out=m2[:],
            in0=m2[:],
            in1=sc[:, 6:8].unsqueeze(2).broadcast_to((K, 2, 32)),
            op=ALU.mult,
        )
        wgt = smallp.tile([K, H, W], F32, tag="wgt")
        nc.vector.tensor_tensor(
            out=wgt[:],
            in0=m2[:, 0:1, :].squeeze(1).unsqueeze(2).broadcast_to((K, H, W)),
            in1=m2[:, 1:2, :].squeeze(1).unsqueeze(1).broadcast_to((K, H, W)),
            op=ALU.mult,
        )
        st["wgt"] = wgt

    def stage_chain_b(s):
        """wgt [K, HW] -> wgtT [128, c, K] via PE transposes. psw comes
        from the psrT pool: its recycling then encodes the real dependency
        (wgtT -> roiT) instead of coupling into the transpose-stream pool.
        The copy runs on ACT: on DVE it would race with (and lose to) the
        next sample's Max/MaxIndex."""
        st = state[s]
        wgt_flat = st["wgt"][:].rearrange("k a b -> k (a b)")
        psw = psrT.tile([P, NDC * K], F32, tag="roiT")
        for c in range(NHWC):
            nc.tensor.transpose(
                out=psw[:, c * K : (c + 1) * K],
                in_=wgt_flat[:, c * P : (c + 1) * P],
                identity=ident_cue[:K, :K],
            )
        wgtT = smallp.tile([P, NHWC, K], F32, tag="wgtT")
        nc.scalar.copy(out=wgtT[:], in_=psw[:, : NHWC * K])
        st["wgtT"] = wgtT

    def stage_roiT(s):
        """roiT[d, k] = sum_hw patches[hw, d] * wgt[k, hw], computed per
        (c, dc) with nat as the STATIONARY operand: each matmul streams only
        K=5 rows -> ~nothing on PE. Out roiT [128, dc, K] in one PSUM bank."""
        st = state[s]
        nat = nat_tiles[s]
        wgtT = st["wgtT"]
        roiT_ps = psrT.tile([P, NDC * K], F32, tag="roiT")
        # fp32 (bitcast from f32r — same bits): HW rejects fp32r matmuls
        # with a 5-wide moving operand (s3d3_mm_fp32r_restrictions).
        # dc OUTER: each dc's accumulation group must fully complete before
        # the next opens — interleaved groups within one PSUM bank corrupt.
        for dc in range(NDC):
            for c in range(NHWC):
                nc.tensor.matmul(
                    out=roiT_ps[:, dc * K : (dc + 1) * K],
                    lhsT=nat[:, c, dc * P : (dc + 1) * P].bitcast(F32),
                    rhs=wgtT[:, c, :],
                    start=(c == 0),
                    stop=(c == NHWC - 1),
                    skip_group_check=True,
                )
        roiT_sb = smallp.tile([P, NDC * K], TRANS_DT, tag="roiTsb")
        nc.scalar.copy(out=roiT_sb[:], in_=roiT_ps[:])
        st["roiT_sb"] = roiT_sb

    def stage_fin(s, half, out_eng=None, on_act=False):
        """Transpose roiT half back to [K, D/2], copy + write out (Pool
        SWDGE queue so the out DMA's waits never block ACT dispatch).
        Mid-stream the PSUM->SBUF copy goes to DVE (ACT is the
        throughput-limiting engine at ~98% of the sample window)."""
        st = state[s]
        roiT_sb = st["roiT_sb"]
        if half == 0:
            out_sb = outp.tile([K, D], F32, tag="outsb")
            st["out_sb"] = out_sb
        else:
            out_sb = st["out_sb"]
        # pst pool (idle at the tail): h0/h1 then don't serialize on one slot
        psfin_t = pst.tile([P, 512], TRANS_DT, tag="pst", name="psfin")
        psfin = psfin_t[:K, :]
        h0, h1 = half * 4, (half + 1) * 4
        for dc in range(h0, h1):
            nc.tensor.transpose(
                out=psfin[:, (dc - h0) * P : (dc - h0 + 1) * P],
                in_=roiT_sb[:, dc * K : (dc + 1) * K],
                identity=ident[:],
            )
        sl = slice(half * 512, (half + 1) * 512)
        if on_act:
            nc.scalar.copy(out=out_sb[:, sl], in_=psfin[:])
        else:
            nc.vector.tensor_copy(out=out_sb[:, sl], in_=psfin[:])
        (out_eng or nc.gpsimd).dma_start(
            out=out_d[s * K : (s + 1) * K, sl], in_=out_sb[:, sl]
        )

    def stage_front(s, prev):
        """Loads (already issued) -> transposes -> copies -> sim matmuls,
        with sample prev's chain/roi ops interleaved at fixed dc points."""
        nat = nat_tiles[s]
        sim_ps = pss.tile([K, HW], F32, tag="sim")
        state[s] = {"sim_ps": sim_ps}

        def sim_mms(dc, pt):
            # Schedule-order hint (build-time only): the Tile list scheduler
            # otherwise hoists each sim matmul into the transpose->copy
            # serial loop, putting its copy-wait on the PE critical cycle.
            # Pushing its readiness past the next few T batches keeps the
            # PE transpose stream free-running at the DMA pace.
            t0 = (2300 + s * WAIT_PER_S) if WAIT_T0 is None else WAIT_T0[s]
            with tc.tile_wait_until(
                (t0 + (dc + WAIT_DCS) * 1456) / 1e6,
                # never delay the LAST sample's sim matmuls: they gate the
                # serial drain chain at the very end of the kernel
                enable=WAIT_DCS > 0,
            ):
                for hf in range(2):
                    nc.tensor.matmul(
                        out=sim_ps[:, hf * 512 : (hf + 1) * 512],
                        lhsT=cueT[:, dc, s * K : (s + 1) * K],
                        rhs=pt[:, hf * 512 : (hf + 1) * 512],
                        start=(dc == 0),
                        stop=(dc == NDC - 1),
                        skip_group_check=True,
                    )

        pending = []
        for dc in range(NDC):
            pt = ptp.tile([P, HW], TRANS_DT, tag="pt")
            for hf in range(2):
                ps = pst.tile([P, 512], TRANS_DT, tag="pst")
                for q in range(4):
                    c = hf * 4 + q
                    nc.tensor.matmul(
                        out=ps[:, q * P : (q + 1) * P],
                        lhsT=nat[:, c, dc * P : (dc + 1) * P],
                        rhs=ident[:],
                        is_transpose=True,
                        skip_group_check=True,
                    )
                dst = pt[:, hf * 512 : (hf + 1) * 512]
                # Optionally pin copies to the ideal load-paced timeline
                # (build-time ordering hint only) to stop scheduler drift
                cpin = (2300 + (s * NDC + dc + 1) * 1456 + COPY_PIN) / 1e6
                with tc.tile_wait_until(cpin, enable=COPY_PIN > 0):
                    # Late-chunk h1 copies on DVE: for the LAST sample the
                    # dc6/dc7 copies sit on the drain-entry critical path
                    # and DVE is idle there — running them in parallel with
                    # ACT's h0 copies pulls sim-stop (and the whole drain)
                    # earlier. Mid-stream DVE is busy with the chain.
                    late = s == NS - 1 and dc >= NDC - 2
                    if late or (hf == 1 and dc >= NDC - DVE_COPY_DCS):
                        if hf == 1:
                            nc.vector.tensor_copy(out=dst, in_=ps[:])
                        else:
                            nc.scalar.copy(out=dst, in_=ps[:])
                    else:
                        nc.scalar.copy(out=dst, in_=ps[:])
            pending.append((dc, pt))
            if len(pending) > LAG_D:
                sim_mms(*pending.pop(0))
        for item in pending:
            sim_mms(*item)
        # Sample prev's roi work rides the PE slack AFTER this sample's sim
        # stream (never in the middle: its DVE/ACT gates would head-of-line
        # block the stream).
        if prev is not None:
            stage_chain_b(prev)
            stage_roiT(prev)
        # argmax chain for THIS sample: issued at the stream tail so the
        # DVE ops start the moment the last sim matmul lands
        stage_chain_a(s)
        # prev's fin AFTER chain_a(s): its DVE copies then queue behind the
        # argmax instead of delaying it. In the LAST iteration they must
        # not touch DVE at all — out-of-order dispatch would slot them
        # between Max and MaxIndex of the final drain chain (ACT is idle
        # there anyway).
        if prev is not None:
            last = s == NS - 1
            stage_fin(prev, 0, on_act=last)
            stage_fin(prev, 1, on_act=last)

    # ---- pipeline across samples ----
    for s in range(NS):
        if s + 1 < NS:
            issue_loads(s + 1)
        stage_front(s, s - 1 if s > 0 else None)
    # epilogue: drain last sample's chain (out DMAs on ACT HWDGE: lower
    # latency than SWDGE and the ACT queue is idle by now).
    # PE keep-warm: the final argmax chain leaves PE idle ~5us, dropping it
    # out of full p-state right before the roiT/fin matmuls. Dummy
    # transposes (no readers) keep the clock up through the gap.
    with tc.tile_wait_until((2300 + (NS - 1) * WAIT_PER_S + WARM_OFF * 1456) / 1e6,
                            enable=N_WARM > 0):
        for i in range(N_WARM):
            psd = pst.tile([P, 512], TRANS_DT, tag="pst", name="warm")
            nc.tensor.transpose(
                out=psd[:, :P],
                in_=nat_tiles[NS - 1][:, 0, :P],
                identity=ident[:],
            )
    stage_chain_b(NS - 1)
    stage_roiT(NS - 1)
    # h0 via Pool SWDGE, h1 via ACT HWDGE: their descriptor-generation
    # stages run on different devices and overlap. h0 copy on ACT and h1
    # copy on DVE (both idle in the drain) so the copies overlap too.
    stage_fin(NS - 1, 0, on_act=True)
    stage_fin(NS - 1, 1, out_eng=nc.scalar, on_act=False)

    ctx.close()


def make_in_maps(cue, patches):
    cue = np.ascontiguousarray(np.asarray(cue, np.float32)).reshape(B, K, D)
    patches = np.ascontiguousarray(np.asarray(patches, np.float32)).reshape(
        B, HW, D
    )
    in_maps = []
    for c in range(NCORES):
        in_maps.append(
            {
                "cue": np.ascontiguousarray(
                    cue[c * NS : (c + 1) * NS].reshape(NS * K, D)
                ),
                "patches": np.ascontiguousarray(
                    patches[c * NS : (c + 1) * NS].reshape(NS * HW, D)
                ),
            }
        )
    return in_maps


_NC_CACHE = None


def get_nc():
    global _NC_CACHE
    if _NC_CACHE is None:
        _NC_CACHE = build_bass()
    return _NC_CACHE


def run(cue, patches, trace=False):
    from concourse.bass_utils import run_bass_kernel_spmd

    nc = get_nc()
    in_maps = make_in_maps(cue, patches)
    res = run_bass_kernel_spmd(
        nc, in_maps, core_ids=list(range(NCORES)), trace=trace
    )
    outs = [r["out"].reshape(NS, K, D) for r in res.results]
    full = np.concatenate(outs, axis=0)
    return full, res


def kernel(cue, patches):
    full, _ = run(cue, patches, trace=False)
    return full
